# revision 7
# baseline (speedup 1.0000x reference)
"""Trainium2 Bass kernel for nn_Decoder_25718264168590.

2-layer LSTM decoder (B=32, T=50, H=1024, E=128) + vocab projection
(V=32000) + log_softmax, distributed over 8 NeuronCores:

- LSTM: gate-sharded 8 ways. Core r owns h-units [r*128, (r+1)*128) of
  both layers (512 gate rows each).  All activations live in
  [feature-on-partition, (ktile, batch)] layout so recurrent matmuls use
  stationary weight tiles and never transpose.  The input projection
  (concat([x_t, enc]) @ proj_w.T) @ w_ih0.T is algebraically folded:
  the x part becomes a 9th K-tile of the layer-0 matmul (A1 = P1 @ w_ih0.T),
  the enc part is a per-core constant added once per step (it is
  t-independent), computed on device-equivalent host fold.
  Per-step h-shards are exchanged with an AllGather over a DRAM bounce.
- Vocab projection: lin_w sharded column-wise (4000 cols/core), resident
  in SBUF; batched matmul over all 1600 samples; log-softmax uses fused
  exp+accumulate on ScalarE plus one small AllReduce per chunk of
  sample tiles for the cross-core sum.  Output rows are written in the
  reference's (b*T + t) order by a strided DMA.
"""

import sys

for _p in ("/opt/trn_rl_repo",):
    if _p not in sys.path:
        sys.path.insert(0, _p)

import numpy as np
import ml_dtypes

B, T, H, E, V = 32, 50, 1024, 128, 32000
NCORES = 8
VS = V // NCORES          # 4000 vocab cols per core
S = B * T                 # 1600 samples, t-major on device: s = t*32 + b
KT = H // 128             # 8 k-tiles of hidden
MT = 4                    # 4 gate m-tiles per core (i, f, o, g)
GPERM = (0, 1, 3, 2)      # torch gate order i,f,g,o -> our col order i,f,o,g
NMT = 13                  # sample m-tiles in vocab phase (12*128 + 64)
NCHK = 8                  # vocab col chunks per core (8 * 500)
CHUNK = VS // NCHK        # 500
AR_CHUNKS = ((0, 4), (4, 8), (8, 11), (11, 13))  # lse AllReduce chunking over m-tiles

BF16 = ml_dtypes.bfloat16

_BUILD_CACHE = {}


def _perm(r, k):
    """h-shard of which core sits in exchange slot k on core r (identity for AG)."""
    return k


def _host_prep(inputs):
    """Fold projections and lay out per-core device arrays."""
    enc = np.asarray(inputs["enc_output"], np.float32)       # (B, H)
    target = np.asarray(inputs["target"], np.float32)        # (B, T, E)
    proj_w = np.asarray(inputs["proj_w"], np.float32)        # (E, H+E)
    proj_b = np.asarray(inputs["proj_b"], np.float32)        # (E,)
    w_ih0 = np.asarray(inputs["w_ih0"], np.float32)          # (4H, E)
    w_hh0 = np.asarray(inputs["w_hh0"], np.float32)          # (4H, H)
    b0 = np.asarray(inputs["b_ih0"], np.float32) + np.asarray(inputs["b_hh0"], np.float32)
    w_ih1 = np.asarray(inputs["w_ih1"], np.float32)          # (4H, H)
    w_hh1 = np.asarray(inputs["w_hh1"], np.float32)          # (4H, H)
    b1 = np.asarray(inputs["b_ih1"], np.float32) + np.asarray(inputs["b_hh1"], np.float32)
    lin_w = np.asarray(inputs["lin_w"], np.float32)          # (V, H)
    lin_b = np.asarray(inputs["lin_b"], np.float32)          # (V,)

    P1 = proj_w[:, :E].T                                     # (E, E)
    P2 = proj_w[:, E:].T                                     # (H, E)
    A1 = P1 @ w_ih0.T                                        # (E, 4H) x-path fold
    genc = (enc @ P2 + proj_b) @ w_ih0.T                     # (B, 4H) enc-path fold

    # t-major input features: xt[e, t*32+b] = target[b, t, e]
    xt = np.ascontiguousarray(
        target.transpose(1, 0, 2).reshape(S, E).T).astype(BF16)          # (128, 1600)

    # exchange-layout encoder init: encx[p, k*32+b] = enc[b, k*128+p]
    encx = np.ascontiguousarray(
        enc.T.reshape(KT, 128, B).transpose(1, 0, 2).reshape(128, KT * B))
    encx_bf = encx.astype(BF16)

    lin_wT = lin_w.T                                         # (H, V)

    in_maps = []
    for r in range(NCORES):
        rows = np.concatenate(
            [np.arange(128) + g * H + r * 128 for g in GPERM])           # 512 gate rows
        m = {}
        m["whh0t"] = np.ascontiguousarray(
            w_hh0[rows].T.reshape(KT, 128, 4 * 128)).astype(BF16)
        m["a1"] = np.ascontiguousarray(A1[:, rows]).astype(BF16)         # (128, 512)
        m["wih1t"] = np.ascontiguousarray(
            w_ih1[rows].T.reshape(KT, 128, 4 * 128)).astype(BF16)
        m["whh1t"] = np.ascontiguousarray(
            w_hh1[rows].T.reshape(KT, 128, 4 * 128)).astype(BF16)
        m["xt"] = xt
        # cg0[p, g*32+b] = genc[b, rows[g*128+p]] + b0[rows[g*128+p]]
        gsel = genc[:, rows] + b0[rows]                                  # (B, 512)
        m["cg0"] = np.ascontiguousarray(
            gsel.T.reshape(MT, 128, B).transpose(1, 0, 2).reshape(128, MT * B))
        m["c1b"] = np.ascontiguousarray(
            np.broadcast_to(b1[rows].reshape(MT, 128, 1),
                            (MT, 128, B)).transpose(1, 0, 2).reshape(128, MT * B)).copy()
        m["encx"] = encx_bf
        m["cinit"] = np.ascontiguousarray(enc.T[r * 128:(r + 1) * 128])  # (128, 32) f32
        lw = lin_wT[:, r * VS:(r + 1) * VS]                              # (H, 4000)
        lwk = np.zeros((KT + 1, 128, VS), np.float32)
        for k in range(KT):
            lwk[k] = lw[_perm(r, k) * 128:(_perm(r, k) + 1) * 128]
        lwk[KT, 0, :] = lin_b[r * VS:(r + 1) * VS]
        m["linwt"] = lwk.astype(BF16)
        in_maps.append(m)
    return in_maps


def _build():
    import concourse.bass as bass
    import concourse.tile as tile
    from concourse import bacc, mybir
    from contextlib import ExitStack

    f32 = mybir.dt.float32
    bf16 = mybir.dt.bfloat16
    AF = mybir.ActivationFunctionType
    ALU = mybir.AluOpType

    nc = bacc.Bacc("TRN2", target_bir_lowering=False, debug=False,
                   num_devices=NCORES)

    d_whh0 = nc.dram_tensor("whh0t", [KT, 128, 512], bf16, kind="ExternalInput")
    d_a1 = nc.dram_tensor("a1", [128, 512], bf16, kind="ExternalInput")
    d_wih1 = nc.dram_tensor("wih1t", [KT, 128, 512], bf16, kind="ExternalInput")
    d_whh1 = nc.dram_tensor("whh1t", [KT, 128, 512], bf16, kind="ExternalInput")
    d_xt = nc.dram_tensor("xt", [128, S], bf16, kind="ExternalInput")
    d_cg0 = nc.dram_tensor("cg0", [128, 128], f32, kind="ExternalInput")
    d_c1b = nc.dram_tensor("c1b", [128, 128], f32, kind="ExternalInput")
    d_encx = nc.dram_tensor("encx", [128, 256], bf16, kind="ExternalInput")
    d_cinit = nc.dram_tensor("cinit", [128, 32], f32, kind="ExternalInput")
    d_linwt = nc.dram_tensor("linwt", [KT + 1, 128, VS], bf16, kind="ExternalInput")
    d_out = nc.dram_tensor("out", [S, VS], f32, kind="ExternalOutput")

    rg = [list(range(NCORES))]

    with tile.TileContext(nc) as tc, ExitStack() as ctx:
        wp = ctx.enter_context(tc.tile_pool(name="w", bufs=1))
        dp = ctx.enter_context(tc.tile_pool(name="db", bufs=3, space="DRAM"))
        hp = ctx.enter_context(tc.tile_pool(name="hx", bufs=2))
        cp = ctx.enter_context(tc.tile_pool(name="ct", bufs=2))
        tp = ctx.enter_context(tc.tile_pool(name="tmp", bufs=3))

        whh0 = [wp.tile([128, 512], bf16, name=f"whh0_{k}") for k in range(KT)]
        a1 = wp.tile([128, 512], bf16, name="a1s")
        wih1 = [wp.tile([128, 512], bf16, name=f"wih1_{k}") for k in range(KT)]
        whh1 = [wp.tile([128, 512], bf16, name=f"whh1_{k}") for k in range(KT)]
        xts = wp.tile([128, S], bf16, name="xts")
        cg0 = wp.tile([128, 128], f32, name="cg0s")
        c1b = wp.tile([128, 128], f32, name="c1bs")
        encx = wp.tile([128, 256], bf16, name="encxs")
        h1store = wp.tile([128, KT * S], bf16, name="h1store")
        linw = [wp.tile([128, VS], bf16, name=f"linw_{k}") for k in range(KT + 1)]
        ones = wp.tile([1, 128], bf16, name="ones")

        for k in range(KT):
            nc.sync.dma_start(whh0[k][:], d_whh0[k])
            nc.sync.dma_start(wih1[k][:], d_wih1[k])
            nc.sync.dma_start(whh1[k][:], d_whh1[k])
        nc.sync.dma_start(a1[:], d_a1[:])
        nc.sync.dma_start(xts[:], d_xt[:])
        nc.sync.dma_start(cg0[:], d_cg0[:])
        nc.sync.dma_start(c1b[:], d_c1b[:])
        nc.sync.dma_start(encx[:], d_encx[:])
        for k in range(KT + 1):
            nc.sync.dma_start(linw[k][:], d_linwt[k])
        nc.gpsimd.memset(ones[:], 1.0)

        ct0 = cp.tile([128, 32], f32, tag="ct0")
        ct1 = cp.tile([128, 32], f32, tag="ct1")
        nc.sync.dma_start(ct0[:], d_cinit[:])
        nc.sync.dma_start(ct1[:], d_cinit[:])

        with tc.tile_pool(name="pg", bufs=2, space="PSUM") as pg:
            h0x_prev = encx

            def h1slice(t, k):
                if t < 0:
                    return encx[:, k * 32:(k + 1) * 32]
                return h1store[:, k * S + t * 32: k * S + t * 32 + 32]

            for t in range(T):
                # ---- layer 0: gates = whh0 @ h0 + A1 @ x_t (+ cg0) ----
                g0 = pg.tile([128, 128], f32, tag="g0")
                for m in range(MT):
                    for k in range(KT + 1):
                        lhsT = whh0[k][:, m * 128:(m + 1) * 128] if k < KT \
                            else a1[:, m * 128:(m + 1) * 128]
                        rhs = h0x_prev[:, k * 32:(k + 1) * 32] if k < KT \
                            else xts[:, t * 32:(t + 1) * 32]
                        nc.tensor.matmul(g0[:, m * 32:(m + 1) * 32], lhsT, rhs,
                                         start=(k == 0), stop=(k == KT))
                nc.vector.tensor_add(g0[:], g0[:], cg0[:])
                sg0 = tp.tile([128, 96], bf16, tag="sg0")
                nc.scalar.activation(sg0[:], g0[:, 0:96], AF.Sigmoid)
                tg0 = tp.tile([128, 32], bf16, tag="tg0")
                nc.scalar.activation(tg0[:], g0[:, 96:128], AF.Tanh)
                t1 = tp.tile([128, 32], f32, tag="t1")
                nc.vector.tensor_mul(t1[:], sg0[:, 0:32], tg0[:])
                ct0n = cp.tile([128, 32], f32, tag="ct0")
                nc.vector.tensor_mul(ct0n[:], sg0[:, 32:64], ct0[:])
                nc.vector.tensor_add(ct0n[:], ct0n[:], t1[:])
                ct0 = ct0n
                th0 = tp.tile([128, 32], bf16, tag="th0")
                nc.scalar.activation(th0[:], ct0[:], AF.Tanh)
                h0s = tp.tile([128, 32], bf16, tag="h0s")
                nc.vector.tensor_mul(h0s[:], sg0[:, 64:96], th0[:])

                # ---- exchange h0 shard -> h0x (AllGather via DRAM bounce) ----
                b0i = dp.tile([128, 32], bf16, tag="b0i")
                b0o = dp.tile([KT, 128, 32], bf16, tag="b0o", addr_space="Shared")
                nc.sync.dma_start(b0i[:], h0s[:])
                nc.gpsimd.collective_compute(
                    "AllGather", ALU.bypass, replica_groups=rg,
                    ins=[b0i[:].opt()], outs=[b0o[:].opt()])
                h0x = hp.tile([128, 256], bf16, tag="h0x")
                nc.sync.dma_start(
                    h0x[:].rearrange("p (k b) -> p k b", k=KT),
                    b0o[:].rearrange("k p b -> p k b"))

                # ---- layer 1: gates = wih1 @ h0_t + whh1 @ h1_{t-1} (+ c1b) ----
                g1 = pg.tile([128, 128], f32, tag="g1")
                for m in range(MT):
                    for k in range(2 * KT):
                        if k < KT:
                            lhsT = wih1[k][:, m * 128:(m + 1) * 128]
                            rhs = h0x[:, k * 32:(k + 1) * 32]
                        else:
                            lhsT = whh1[k - KT][:, m * 128:(m + 1) * 128]
                            rhs = h1slice(t - 1, k - KT)
                        nc.tensor.matmul(g1[:, m * 32:(m + 1) * 32], lhsT, rhs,
                                         start=(k == 0), stop=(k == 2 * KT - 1))
                nc.vector.tensor_add(g1[:], g1[:], c1b[:])
                sg1 = tp.tile([128, 96], bf16, tag="sg1")
                nc.scalar.activation(sg1[:], g1[:, 0:96], AF.Sigmoid)
                tg1 = tp.tile([128, 32], bf16, tag="tg1")
                nc.scalar.activation(tg1[:], g1[:, 96:128], AF.Tanh)
                t2 = tp.tile([128, 32], f32, tag="t2")
                nc.vector.tensor_mul(t2[:], sg1[:, 0:32], tg1[:])
                ct1n = cp.tile([128, 32], f32, tag="ct1")
                nc.vector.tensor_mul(ct1n[:], sg1[:, 32:64], ct1[:])
                nc.vector.tensor_add(ct1n[:], ct1n[:], t2[:])
                ct1 = ct1n
                th1 = tp.tile([128, 32], bf16, tag="th1")
                nc.scalar.activation(th1[:], ct1[:], AF.Tanh)
                h1s = tp.tile([128, 32], bf16, tag="h1s")
                nc.vector.tensor_mul(h1s[:], sg1[:, 64:96], th1[:])

                # ---- exchange h1 shard -> h1store[:, k*S + t*32 ...] ----
                b1i = dp.tile([128, 32], bf16, tag="b1i")
                b1o = dp.tile([KT, 128, 32], bf16, tag="b1o", addr_space="Shared")
                nc.sync.dma_start(b1i[:], h1s[:])
                nc.gpsimd.collective_compute(
                    "AllGather", ALU.bypass, replica_groups=rg,
                    ins=[b1i[:].opt()], outs=[b1o[:].opt()])
                nc.sync.dma_start(
                    h1store[:].rearrange("p (k s) -> p k s", k=KT)
                    [:, :, t * 32:(t + 1) * 32],
                    b1o[:].rearrange("k p b -> p k b"))
                h0x_prev = h0x

        # ================= vocab projection + log_softmax =================
        with tc.tile_pool(name="vp", bufs=4, space="PSUM") as vp, \
             tc.tile_pool(name="lg", bufs=5) as lgp, \
             tc.tile_pool(name="sm", bufs=3) as smp, \
             tc.tile_pool(name="ob", bufs=2) as obp, \
             tc.tile_pool(name="ex", bufs=3) as exp_p, \
             tc.tile_pool(name="tot", bufs=1) as totp:

            totals = totp.tile([128, 16], f32, name="totals")
            lse = totp.tile([128, 16], f32, name="lse")
            neglse = totp.tile([128, 16], f32, name="neglse")
            out_tb = d_out[:].rearrange("(b t) v -> t b v", b=B)

            lgt = {}
            for (c0, c1) in AR_CHUNKS:
                for m in range(c0, c1):
                    M = 128 if m < NMT - 1 else S - 128 * (NMT - 1)
                    lg = lgp.tile([128, VS], bf16, tag="lg")
                    lgt[m] = (lg, M)
                    sm = smp.tile([128, NCHK], f32, tag="sm")
                    for c in range(NCHK):
                        ps = vp.tile([128, CHUNK], f32, tag="ps")
                        for k in range(KT + 1):
                            if k < KT:
                                lhsT = h1store[:, k * S + m * 128:
                                               k * S + m * 128 + M]
                            else:
                                lhsT = ones[0:1, 0:M]
                            rhs = linw[k][:, c * CHUNK:(c + 1) * CHUNK] if k < KT \
                                else linw[k][0:1, c * CHUNK:(c + 1) * CHUNK]
                            nc.tensor.matmul(ps[:M], lhsT, rhs,
                                             start=(k == 0), stop=(k == KT))
                        nc.vector.tensor_copy(lg[:M, c * CHUNK:(c + 1) * CHUNK],
                                              ps[:M])
                        ex = exp_p.tile([128, CHUNK], bf16, tag="ex")
                        nc.scalar.activation(ex[:M], ps[:M], AF.Exp,
                                             accum_out=sm[:M, c:c + 1])
                    nc.vector.tensor_reduce(totals[:M, m:m + 1], sm[:M, :],
                                            axis=mybir.AxisListType.X, op=ALU.add)

                # one AllReduce for this chunk's per-sample exp-sums
                nm = c1 - c0
                ari = dp.tile([128, nm], f32, tag="ari")
                aro = dp.tile([128, nm], f32, tag="aro", addr_space="Shared")
                nc.sync.dma_start(ari[:], totals[:, c0:c1])
                nc.gpsimd.collective_compute(
                    "AllReduce", ALU.add, replica_groups=rg,
                    ins=[ari[:].opt()], outs=[aro[:].opt()])
                nc.sync.dma_start(lse[:, c0:c1], aro[:])
                nc.scalar.activation(lse[:, c0:c1], lse[:, c0:c1], AF.Ln)
                nc.vector.tensor_scalar_mul(neglse[:, c0:c1], lse[:, c0:c1], -1.0)

                for m in range(c0, c1):
                    lg, M = lgt.pop(m)
                    q = M // 32
                    for h in range(2):
                        ob = obp.tile([128, VS // 2], f32, tag="ob")
                        if (m + h) % 2 == 0:
                            nc.vector.tensor_scalar(
                                ob[:M], lg[:M, h * (VS // 2):(h + 1) * (VS // 2)],
                                lse[:M, m:m + 1], None, op0=ALU.subtract)
                        else:
                            nc.scalar.activation(
                                ob[:M], lg[:M, h * (VS // 2):(h + 1) * (VS // 2)],
                                AF.Identity, bias=neglse[:M, m:m + 1])
                        dst = out_tb[m * 4:m * 4 + q, :,
                                     h * (VS // 2):(h + 1) * (VS // 2)]
                        nc.sync.dma_start(dst, ob[:M])

    nc.compile()
    return nc


def _get_nc():
    if "nc" not in _BUILD_CACHE:
        _BUILD_CACHE["nc"] = _build()
    return _BUILD_CACHE["nc"]


def run(inputs, trace=False):
    from concourse.bass_utils import run_bass_kernel_spmd

    in_maps = _host_prep(inputs)
    nc = _get_nc()
    res = run_bass_kernel_spmd(nc, in_maps, core_ids=list(range(NCORES)),
                               trace=trace)
    full = np.empty((S, V), np.float32)
    for r in range(NCORES):
        full[:, r * VS:(r + 1) * VS] = res.results[r]["out"]
    return full, res


def kernel(**inputs):
    full, _ = run(inputs)
    return full
